# revision 37
# baseline (speedup 1.0000x reference)
"""Causal sliding-window attention (B=2, T=2048, D=1024, H=16, W=512) on 8 trn2 cores.

Sequence-parallel sharding: each core owns 512 consecutive tokens of one batch
and recomputes the 512-token halo k/v locally (no collectives). All compute is
feature-major (tokens on the matmul free dim) in float32r.

v3: engine-balanced pipeline, DMA-stream-ordered startup.
  - DMA transfers are serial (~360 B/ns) in the machine, so transfers are
    issued in first-use order at per-kd granularity for the q-proj gate:
    x_own/wq interleaved, then wv, x_halo, wk, wo. Phases run q -> v -> k so
    k's staggered co-outer evictions feed straight into attention.
  - attention: 8 key-tiles per head packed into 4 psum score groups
    (A=kb1|kb0, B=kb3|kb2, C=kb4|kb5, D=Z|kb7|kb6) with query-aligned column
    layouts; one exp per group (4 ACT instrs/head), one fused band-mask
    multiply per group (strided/repeated APs, 4 DVE instrs/head); padded attV
    windows read permanently zeroed pt columns (Pool memset on a bitcast f32
    view; f32r cannot be memset directly).
  - softmax: a ones column rides in the attV stationary; even heads use
    [v|ones] -> psum rows 0:65, odd heads [zeros63|ones|v] -> rows 63:128, so
    the normalized pair lands on partitions 0:64 / 64:128 of one attT tile
    with no cross-partition copy. DVE reciprocal -> gpsimd partition_broadcast
    -> DVE multiply, all partition-base aligned.
  - out-proj: head-pair contraction K=128, 8 matmuls per output tile; wo rows
    for pair hp are the contiguous block [128*hp, 128*hp+128).
"""
import sys

sys.path.insert(0, "/opt/trn_rl_repo")

import numpy as np

B, T, D = 2, 2048, 1024
H, HD, W = 16, 64, 512
NCORES = 8
CHUNK = 512  # own tokens per core
TOK = 2 * CHUNK  # halo + own
NKD = D // 128  # 8 contraction tiles
SCALE = HD ** -0.5


_BUILT = None


def _build():
    import concourse.bass as bass
    import concourse.tile as tile
    from concourse import mybir, bacc

    f32 = mybir.dt.float32
    f32r = mybir.dt.float32r

    nc = bacc.Bacc("TRN2", target_bir_lowering=False, debug=False,
                   num_devices=NCORES)
    xT = nc.dram_tensor("xT", [D, TOK], f32r, kind="ExternalInput")
    wq = nc.dram_tensor("wq", [D, D], f32r, kind="ExternalInput")
    wk = nc.dram_tensor("wk", [D, D], f32r, kind="ExternalInput")
    wv = nc.dram_tensor("wv", [D, D], f32r, kind="ExternalInput")
    wo = nc.dram_tensor("wo", [D, D], f32r, kind="ExternalInput")
    # [:, 0, :] = strict-lower-tri (halo diag blocks), [:, 1, :] = upper-incl
    mask = nc.dram_tensor("mask", [128, 2, 128], f32, kind="ExternalInput")
    # exp bias per score group (A,B = halo: -250 on chunk-0 cores; C,D = 0)
    kbias = nc.dram_tensor("kbias", [128, 4], f32, kind="ExternalInput")
    outT = nc.dram_tensor("outT", [D, CHUNK], f32, kind="ExternalOutput")

    xT_r = xT.rearrange("(kd p) t -> kd p t", p=128)
    w_r = {n: w.rearrange("(kd p) c -> kd p c", p=128)
           for n, w in (("wq", wq), ("wk", wk), ("wv", wv))}
    # wo with head-pair rows contiguous: partition p of pair hp = wo row
    # 128*hp + p (head 2hp dims at p<64, head 2hp+1 dims at p>=64)
    wo_r = wo.rearrange("(hp p) e -> p hp e", p=128)

    with tile.TileContext(nc) as tc:
        with tc.tile_pool(name="const", bufs=1) as constp, \
             tc.tile_pool(name="qkv", bufs=1) as qkvp:

            mask_sb = constp.tile([128, 2, 128], f32)
            nc.sync.dma_start(out=mask_sb, in_=mask[:, :, :])
            kbias_sb = constp.tile([128, 4], f32)
            nc.sync.dma_start(out=kbias_sb, in_=kbias[:, :])

            # ---- persistent qkv buffers (feature-major q/k, token-major v)
            qT_sb = qkvp.tile([128, NKD, CHUNK], f32r)   # q dims x own tokens
            kT_sb = qkvp.tile([128, NKD, TOK], f32r)     # k dims x keys
            # v stationaries, interleaved [ones64 | v_odd64 | v_even64] x 64
            # segments + one final ones block, so that per (key-tile, pair):
            #   odd slot  = [ones|v]  (cols 192i   .. 192i+128, contiguous)
            #   even slot = [v|ones]  (cols 192i+128 .. 192i+256, contiguous)
            # Both are single-free-dim 128-col stationaries (an LDWEIGHTS
            # requirement); the ones half makes attV emit softmax sums
            # replicated on 64 psum partitions (even: rows 64:128, odd: rows
            # 0:64), so the normalize is a recip + one mixed-base multiply
            # with no partition broadcast at all.
            NSEG = NKD * (H // 2)
            v_sb = qkvp.tile([128, NSEG * 192 + 64], f32r)
            # small final ones block first: it feeds the PE warm-up chain
            nc.gpsimd.memset(v_sb[:, NSEG * 192:].bitcast(f32), 1.0)
            ones_view = v_sb[:, 0:NSEG * 192].rearrange(
                "p (i seg) -> p i seg", seg=192)
            nc.gpsimd.memset(ones_view[:, :, 0:64].bitcast(f32), 1.0)
            vv = v_sb[:, 0:NSEG * 192].rearrange(
                "p (kb hp seg) -> p kb hp seg", hp=H // 2, seg=192)

            with tc.tile_pool(name="wx", bufs=1) as wxp, \
                 tc.tile_pool(name="qkv_ps", bufs=1, space="PSUM") as qps:
                x_own = wxp.tile([128, NKD, CHUNK], f32r)
                x_halo = wxp.tile([128, NKD, CHUNK], f32r)
                # wk reuses wq's buffers (k-proj runs last; its DMAs wait for
                # q-proj's final reads); wv gets its own, prefetchable pair.
                w_sb = {wn: [wxp.tile([128, 4, D], f32r, name=f"{wn}_{half}",
                             tag=f"{tg}{half}")
                             for half in range(2)]
                        for wn, tg in (("wq", "wqk"), ("wv", "wv"))}

                # DMA issue order == first-use order (transfers are serial,
                # ~360 B/ns): per-kd x_own/wq pairs gate q-proj start at ~3us;
                # wv and x_halo stream behind during q/v compute; wk last.
                for kd in range(NKD):
                    nc.sync.dma_start(out=x_own[:, kd, :],
                                      in_=xT_r[kd, :, CHUNK:TOK])
                    nc.sync.dma_start(out=w_sb["wq"][kd // 4][:, kd % 4, :],
                                      in_=w_r["wq"][kd])
                for kd in range(NKD):
                    nc.sync.dma_start(out=w_sb["wv"][kd // 4][:, kd % 4, :],
                                      in_=w_r["wv"][kd])
                for kd in range(NKD):
                    nc.sync.dma_start(out=x_halo[:, kd, :],
                                      in_=xT_r[kd, :, 0:CHUNK])

                def wt(wn, kd):
                    return w_sb[wn][kd // 4][:, kd % 4, :]

                # ---- PE warm-up: dummy matmuls on the ones block while the
                # first x/wq DMAs land, so real matmuls start at full p-state
                # (the PE needs ~3us of continuous work to leave low p-state)
                warm = qps.tile([128, CHUNK], f32, name="warm", tag="ps7")
                ob = v_sb[:, NSEG * 192:NSEG * 192 + 64]
                for _ in range(58):
                    nc.tensor.matmul(warm[0:64, 0:64], ob, ob,
                                     start=True, stop=True)

                # ---- q projection, kd-outer: 8 parallel psum chains
                ps_q = [qps.tile([128, CHUNK], f32, name=f"psq{co}",
                                 tag=f"ps{co}") for co in range(NKD)]
                for kd in range(NKD):
                    for co in range(NKD):
                        nc.tensor.matmul(
                            ps_q[co][:],
                            wt("wq", kd)[:, co * 128:(co + 1) * 128],
                            x_own[:, kd, :],
                            start=(kd == 0), stop=(kd == NKD - 1))
                for co in range(NKD):
                    nc.scalar.copy(qT_sb[:, co, :], ps_q[co][:])

                # wk loads into wq's (now dead) buffers
                w_sb["wk"] = [wxp.tile([128, 4, D], f32r, name=f"wk_{half}",
                                       tag=f"wqk{half}")
                              for half in range(2)]
                for kd in range(NKD):
                    nc.sync.dma_start(out=w_sb["wk"][kd // 4][:, kd % 4, :],
                                      in_=w_r["wk"][kd])

                # ---- v projection (own token tiles first, halo second),
                # kd-outer groups of 8 chains; v[tt] = xT[:,tt].T @ wv
                for grp in (range(4, 8), range(0, 4)):
                    ps_v = {(tt, cv): qps.tile(
                        [128, CHUNK], f32, name=f"psv{cv}{tt}",
                        tag=f"ps{(tt % 4) * 2 + cv}")
                        for tt in grp for cv in range(2)}
                    for kd in range(NKD):
                        for tt in grp:
                            xs = x_halo if tt < 4 else x_own
                            tl = (tt % 4) * 128
                            for cv in range(2):
                                nc.tensor.matmul(
                                    ps_v[tt, cv][:],
                                    xs[:, kd, tl:tl + 128],
                                    wt("wv", kd)[:, cv * CHUNK:(cv + 1) * CHUNK],
                                    start=(kd == 0), stop=(kd == NKD - 1))
                    for tt in grp:
                        for cv in range(2):
                            # scatter 8 heads (cols of 64) into v stationaries
                            ps4 = ps_v[tt, cv][:].rearrange(
                                "p (g par d) -> p g par d", par=2, d=HD)
                            g0 = cv * 4
                            nc.scalar.copy(
                                vv[:, tt, g0:g0 + 4, 128:192],
                                ps4[:, :, 0, :])
                            nc.scalar.copy(
                                vv[:, tt, g0:g0 + 4, 64:128],
                                ps4[:, :, 1, :])

                # ---- k projection: own tokens kd-outer (wk still arriving),
                # then halo co-outer so evictions stagger into attention
                ps_k = [qps.tile([128, CHUNK], f32, name=f"psk1{co}",
                                 tag=f"ps{co}") for co in range(NKD)]
                for kd in range(NKD):
                    for co in range(NKD):
                        nc.tensor.matmul(
                            ps_k[co][:],
                            wt("wk", kd)[:, co * 128:(co + 1) * 128],
                            x_own[:, kd, :],
                            start=(kd == 0), stop=(kd == NKD - 1))
                for co in range(NKD):
                    nc.scalar.copy(kT_sb[:, co, CHUNK:TOK], ps_k[co][:])
                for co in range(NKD):
                    ps = qps.tile([128, CHUNK], f32, name=f"psk0{co}",
                                  tag=f"ps{co}")
                    for kd in range(NKD):
                        nc.tensor.matmul(
                            ps[:],
                            wt("wk", kd)[:, co * 128:(co + 1) * 128],
                            x_halo[:, kd, :],
                            start=(kd == 0), stop=(kd == NKD - 1))
                    nc.scalar.copy(kT_sb[:, co, 0:CHUNK], ps[:])

            # ---- attention + output projection
            with tc.tile_pool(name="attb", bufs=1) as attbp, \
                 tc.tile_pool(name="nrm", bufs=2) as nrmp, \
                 tc.tile_pool(name="oev", bufs=4) as oevp:

                # attT pair layout: partitions 0:64 = even head dims,
                # 64:128 = odd head dims; slot hp = head pair
                attT = attbp.tile([128, NKD, CHUNK], f32r)

                # wo loads overlap attention (x/w space freed above)
                wo_sb = attbp.tile([128, NKD, D], f32r)
                for half in range(2):
                    nc.sync.dma_start(
                        out=wo_sb[:, 4 * half:4 * half + 4, :],
                        in_=wo_r[:, 4 * half:4 * half + 4, :])

                # persistent, manually double-buffered pt tiles; zero-pad
                # columns (ptA[384:512], ptD[0:128]) are written once and
                # only ever read afterwards.
                ptA = [attbp.tile([128, 512], f32r, name=f"ptA{i}")
                       for i in range(2)]
                ptB = [attbp.tile([128, 896], f32r, name=f"ptB{i}")
                       for i in range(2)]
                ptC = [attbp.tile([128, 896], f32r, name=f"ptC{i}")
                       for i in range(2)]
                ptD = [attbp.tile([128, 512], f32r, name=f"ptD{i}")
                       for i in range(2)]
                for i in range(2):
                    nc.vector.memset(ptA[i][:, 384:512].bitcast(f32), 0.0)
                    nc.vector.memset(ptD[i][:, 0:128].bitcast(f32), 0.0)

                def rep_mask(plane):
                    # [128, 2, 128] view of one mask plane repeated twice
                    base = mask_sb[:, plane, :]
                    return bass.AP(tensor=base.tensor, offset=base.offset,
                                   ap=[list(base.ap[0]), [0, 2], [1, 128]])

                def two_blocks(t, off, stride):
                    # [128, 2, 128] strided view: cols [off:off+128] and
                    # [off+stride:off+stride+128] of tile t
                    base = t[:, off:off + 128]
                    return bass.AP(tensor=base.tensor, offset=base.offset,
                                   ap=[list(base.ap[0]), [stride, 2],
                                       [1, 128]])

                attention_scope = tc.tile_pool(name="ps_sc", bufs=1,
                                               space="PSUM")
                ps_sc = attention_scope.__enter__()
                at_scope = tc.tile_pool(name="ps_at", bufs=2, space="PSUM")
                ps_at = at_scope.__enter__()

                def emit_scores(h):
                    # ---- scores: 4 psum groups, query-aligned columns
                    hp, po = h // 2, (h % 2) * 64
                    kt = kT_sb[po:po + 64, hp, :]
                    qt = qT_sb[po:po + 64, hp, :]
                    scA = ps_sc.tile([128, 512], f32, tag="scA")
                    scB = ps_sc.tile([128, 1024], f32, tag="scB")
                    scC = ps_sc.tile([128, 1024], f32, tag="scC")
                    scD = ps_sc.tile([128, 512], f32, tag="scD")
                    # B: kb3 q[0:512) at cols 0:512, kb2 q[0:384) at 512:896
                    nc.tensor.matmul(scB[:, 0:512], kt[:, 384:512],
                                     qt[:, 0:512], start=True, stop=True)
                    nc.tensor.matmul(scB[:, 512:896], kt[:, 256:384],
                                     qt[:, 0:384], start=True, stop=True)
                    # C: kb4 q[0:512) at cols 0:512, kb5 q[128:512) at 512:896
                    nc.tensor.matmul(scC[:, 0:512], kt[:, 512:640],
                                     qt[:, 0:512], start=True, stop=True)
                    nc.tensor.matmul(scC[:, 512:896], kt[:, 640:768],
                                     qt[:, 128:512], start=True, stop=True)
                    # A: kb1 q[0:256) at cols 0:256, kb0 q[0:128) at 256:384
                    # (256:512 written, 384:512 is dead padding)
                    nc.tensor.matmul(scA[:, 0:256], kt[:, 128:256],
                                     qt[:, 0:256], start=True, stop=True)
                    nc.tensor.matmul(scA[:, 256:512], kt[:, 0:128],
                                     qt[:, 0:256], start=True, stop=True)
                    # D: kb7 q[384:512) at cols 128:256 (0:256 written, real
                    # at 128:256), kb6 q[256:512) at cols 256:512
                    nc.tensor.matmul(scD[:, 0:256], kt[:, 896:1024],
                                     qt[:, 256:512], start=True, stop=True)
                    nc.tensor.matmul(scD[:, 256:512], kt[:, 768:896],
                                     qt[:, 256:512], start=True, stop=True)
                    return scA, scB, scC, scD

                sc_next = emit_scores(0)
                for h in range(H):
                    hp, odd = h // 2, h % 2
                    po = odd * 64
                    buf = h % 2
                    scA, scB, scC, scD = sc_next

                    # ---- exp (one per group) + fused band masks (B/C masks
                    # run on the gpsimd engine to keep the DVE under the ACT
                    # pacing budget)
                    EXP = mybir.ActivationFunctionType.Exp
                    nc.scalar.activation(ptB[buf][:, 0:896], scB[:, 0:896],
                                         EXP, bias=kbias_sb[:, 1:2],
                                         scale=SCALE)
                    nc.gpsimd.tensor_mul(two_blocks(ptB[buf], 384, 384),
                                         two_blocks(ptB[buf], 384, 384),
                                         rep_mask(0))
                    nc.scalar.activation(ptC[buf][:, 0:896], scC[:, 0:896],
                                         EXP, bias=kbias_sb[:, 2:3],
                                         scale=SCALE)
                    nc.gpsimd.tensor_mul(two_blocks(ptC[buf], 0, 512),
                                         two_blocks(ptC[buf], 0, 512),
                                         rep_mask(1))
                    nc.scalar.activation(ptA[buf][:, 0:384], scA[:, 0:384],
                                         EXP, bias=kbias_sb[:, 0:1],
                                         scale=SCALE)
                    nc.vector.tensor_mul(ptA[buf][:, 128:384],
                                         ptA[buf][:, 128:384],
                                         rep_mask(0))
                    nc.scalar.activation(ptD[buf][:, 128:512], scD[:, 128:512],
                                         EXP, bias=kbias_sb[:, 3:4],
                                         scale=SCALE)
                    nc.vector.tensor_mul(ptD[buf][:, 128:384],
                                         ptD[buf][:, 128:384],
                                         rep_mask(1))

                    # prefetch next head's scores (PE) ahead of this head's
                    # exp-gated attV so the PE never waits on the ACT engine
                    if h + 1 < H:
                        sc_next = emit_scores(h + 1)

                    # ---- attV: two-block stationaries put att on one psum
                    # half and the softmax sums (replicated x64) on the other
                    att_ps = ps_at.tile([128, CHUNK], f32, tag="att")

                    def vst(kb):
                        off = (kb * (H // 2) + hp) * 192 + 128 * (1 - odd)
                        return v_sb[:, off:off + 128]

                    nc.tensor.matmul(att_ps[0:128, 0:512], vst(3),
                                     ptB[buf][:, 0:512],
                                     start=True, stop=False)
                    nc.tensor.matmul(att_ps[0:128, 0:384], vst(2),
                                     ptB[buf][:, 512:896],
                                     start=False, stop=False)
                    nc.tensor.matmul(att_ps[0:128, 0:512], vst(4),
                                     ptC[buf][:, 0:512],
                                     start=False, stop=False)
                    nc.tensor.matmul(att_ps[0:128, 128:512], vst(5),
                                     ptC[buf][:, 512:896],
                                     start=False, stop=False)
                    nc.tensor.matmul(att_ps[0:128, 0:256], vst(1),
                                     ptA[buf][:, 0:256],
                                     start=False, stop=False)
                    nc.tensor.matmul(att_ps[0:128, 0:256], vst(0),
                                     ptA[buf][:, 256:512],
                                     start=False, stop=False)
                    nc.tensor.matmul(att_ps[0:128, 256:512], vst(7),
                                     ptD[buf][:, 0:256],
                                     start=False, stop=False)
                    nc.tensor.matmul(att_ps[0:128, 256:512], vst(6),
                                     ptD[buf][:, 256:512],
                                     start=False, stop=True)

                    # ---- normalize: reciprocal of the replicated sums half,
                    # then one psum-evicting multiply (mixed partition bases
                    # are legal because in0 is PSUM)
                    rt = nrmp.tile([128, CHUNK], f32, tag="rt")
                    so = 64 - po  # sums half is the opposite 64 partitions
                    nc.vector.reciprocal(rt[so:so + 64, :],
                                         att_ps[so:so + 64, :])
                    nc.vector.tensor_mul(attT[po:po + 64, hp, :],
                                         att_ps[po:po + 64, :],
                                         rt[so:so + 64, :])

                at_scope.__exit__(None, None, None)
                attention_scope.__exit__(None, None, None)

                # ---- output projection: 8 K=128 pair-matmuls per out tile
                with tc.tile_pool(name="ps_o", bufs=3, space="PSUM") as ps_o:
                    for eo in range(NKD):
                        ps = ps_o.tile([128, CHUNK], f32, tag="op")
                        for hp in range(NKD):
                            nc.tensor.matmul(
                                ps[:],
                                wo_sb[:, hp, eo * 128:(eo + 1) * 128],
                                attT[:, hp, :],
                                start=(hp == 0), stop=(hp == NKD - 1))
                        ot = oevp.tile([128, CHUNK], f32, tag="ot")
                        # alternate eviction engines so copies overlap
                        if eo % 2 == 0:
                            nc.scalar.copy(ot[:], ps[:])
                        else:
                            nc.vector.tensor_copy(ot[:], ps[:])
                        nc.sync.dma_start(out=outT[eo * 128:(eo + 1) * 128, :],
                                          in_=ot[:])

    nc.compile()
    return nc


def _host_inputs(x, w_qkv, w_out):
    x = np.ascontiguousarray(np.asarray(x, dtype=np.float32))
    w_qkv = np.ascontiguousarray(np.asarray(w_qkv, dtype=np.float32))
    w_out = np.ascontiguousarray(np.asarray(w_out, dtype=np.float32))

    wq = np.ascontiguousarray(w_qkv[:, 0:D])
    wk = np.ascontiguousarray(w_qkv[:, D:2 * D])
    wv = np.ascontiguousarray(w_qkv[:, 2 * D:3 * D])

    r = np.arange(128)[:, None]
    c = np.arange(128)[None, :]
    mask = np.zeros((128, 2, 128), dtype=np.float32)
    mask[:, 0, :] = (r > c).astype(np.float32)   # halo diag blocks
    mask[:, 1, :] = (r <= c).astype(np.float32)  # own diag blocks

    in_maps = []
    for core in range(NCORES):
        b, qc = divmod(core, 4)
        q0 = qc * CHUNK
        xa = np.zeros((TOK, D), dtype=np.float32)
        lo = max(0, q0 - CHUNK)
        xa[CHUNK - (q0 - lo):] = x[b, lo:q0 + CHUNK]
        kb_bias = np.zeros((128, 4), dtype=np.float32)
        if qc == 0:
            kb_bias[:, 0:2] = -250.0  # groups A,B cover the (zero) halo keys
        in_maps.append({
            "xT": np.ascontiguousarray(xa.T),
            "wq": wq, "wk": wk, "wv": wv, "wo": w_out,
            "mask": mask, "kbias": kb_bias,
        })
    return in_maps


def kernel(x, w_qkv, w_out):
    global _BUILT
    if _BUILT is None:
        _BUILT = _build()
    from concourse.bass_utils import run_bass_kernel_spmd

    in_maps = _host_inputs(x, w_qkv, w_out)
    res = run_bass_kernel_spmd(_BUILT, in_maps, core_ids=list(range(NCORES)))
    out = np.empty((B, T, D), dtype=np.float32)
    for core in range(NCORES):
        b, qc = divmod(core, 4)
        out[b, qc * CHUNK:(qc + 1) * CHUNK, :] = res.results[core]["outT"].T
    return out
